# revision 9
# baseline (speedup 1.0000x reference)
"""Trainium2 Bass kernel for nn_MultiHeadedAttention_41583873359904.

Reference computation (B=8, C=256, H=W=128):
  q/k/v = 1x1 conv projections of x/y/z
  scores[b,c,h,h'] = q[b,c,h,:].k[b,c,h',:]/sqrt(W); p = softmax(scores, -1)
  att = p @ v  (per b,c)
  o = conv3x3(att) + b_out -> BatchNorm2d(batch stats) -> LeakyReLU(0.2)

Sharding: data-parallel over batch, one batch element per NeuronCore (8 cores).
BatchNorm batch stats are combined with an on-device AllReduce of per-core
(sum, sumsq) so the whole computation is a single NEFF.

Bias handling (all exact):
  - b_out cancels through BatchNorm (constant per channel, subtracted by mean).
  - bk adds only h-dependent terms to scores -> cancels in softmax over h'.
  - bq adds bq*Sk(h') to scores (Sk = sum_w k~): folded multiplicatively as
    m[h'] = exp(bq*Sk[h']); att = (E @ (m*v)) / (E @ m). Sk comes from one
    N=1 matmul per channel against a bq-replicated column, m from one Exp.
  - bv is applied in the V projection via the activation bias operand.

Per-core layout strategy (DMAs are batched aggressively -- each dma_start
costs ~600ns on the shared descriptor generator regardless of size):
  - V projection channel-major [oc, pix] -> v_dram[C, HW]; attention reads it
    back as [h', 16ch, w] gather tiles (one DMA per 16 channels).
  - Q/K projections pixel-major -> Q_sb/K_sb [w, h, c] in SBUF (contraction
    over w needs w on partitions); scores^T = K^T.T @ Q^T per channel,
    Exp on ACT, att = E^T.T @ (m*v) with an m-column matmul giving the
    softmax denominator; normalize folded into the PSUM->SBUF copy.
  - att planes written zero-padded [C,130,130] to DRAM in 16-channel blocks;
    3x3 conv = 18 accumulated matmuls (2 ic chunks x 9 taps) per
    [128oc, 512pix] PSUM tile with shifted access patterns.

Matmul operands are bf16 (fp32 PSUM accumulation).
"""

import math

import numpy as np
import ml_dtypes

import concourse.bass as bass
import concourse.tile as tile
from concourse import mybir
from concourse import tile_sem_assignment as _tsa
from concourse.tile import ScopedClock as _ScopedClock
from concourse.bass_utils import run_bass_kernel_spmd

B, C, H, W = 8, 256, 128, 128
HW = H * W          # 16384 pixels per plane
PB = 512            # pixels per conv/proj psum tile (4 rows)
NB = HW // PB       # 32 pixel blocks
CH = C // 128       # 2 channel chunks of 128
GC = 16             # channels per attention group
BN_EPS = 1e-5
LEAKY = 0.2
N_CORES = 8
N_TOT = float(B * HW)   # BN element count per channel

VB = 2048           # pixels per V-phase chunk (8 chunks)
QB = 2048           # pixels per QK-phase chunk (16 h-rows)
CB = 4              # conv pb blocks per load (16 h-rows)

BF16 = mybir.dt.bfloat16
F32 = mybir.dt.float32
nbf16 = ml_dtypes.bfloat16


class _SplitDrainTileContext(tile.TileContext):
    """The walrus in this container rejects >1 sync wait per instruction.
    Tile routinely emits several (RAW + WAR). Hoist extra waits onto NOPs
    committed immediately before on the same engine (sequencers execute in
    order, so waiting on the NOPs first is equivalent), and split the tail
    drain's global-clock waits the same way."""

    def _commit_instruction(self, inst, lazy_reg_writes=True):
        si = getattr(inst, "sync_info", None)
        if (
            si is not None
            and si.on_wait
            and len(si.on_wait) > 1
            and inst.engine != mybir.EngineType.Unassigned
            and not isinstance(inst, mybir.InstUnconditionalBranch)
        ):
            waits = list(si.on_wait)
            for w in waits[:-1]:
                nop = mybir.InstNoOp(
                    name=self.nc.get_next_instruction_name(),
                    engine=inst.engine,
                    ins=[],
                    outs=[],
                    sync_info=mybir.SyncInfo(on_wait=[w], on_update=[]),
                    bass_nofuse=True,
                )
                super()._commit_instruction(nop, lazy_reg_writes=False)
            inst.sync_info = mybir.SyncInfo(
                on_wait=[waits[-1]], on_update=list(si.on_update or [])
            )
        super()._commit_instruction(inst, lazy_reg_writes)

    def _drain_and_barrier(self, tick_clock, wait_clock):
        nc = self.nc
        gc = tick_clock.global_clock
        procs = [(p, gc.peek_next(p) - 1) for p in range(_tsa.N_PROCS)]
        for p, t in procs:
            if t <= 0:
                continue
            sub = _tsa.VectorClock()
            sub.require_at_least(p, t)
            nop = nc.sync.nop(nofuse=True, hint="split_drain_wait")
            wait_clock.add_sem_waits(nop.ins, _ScopedClock({None: sub}))
        nc.sync.drain()
        nc.all_engine_barrier()
        assert self.sems is not None
        popped = nc._tile_sem_poison_stack.pop()
        assert popped is self._sem_poison
        nc.clear_and_free_semaphores(list(self.sems.allocated().values()))
        nc.all_engine_barrier()


_PHASE_MARKS = []


def _mark(nc, name):
    _PHASE_MARKS.append((name, int(nc.get_next_instruction_name()[2:])))


def _build(sim=False):
    _PHASE_MARKS.clear()
    nc = bass.Bass(num_devices=N_CORES)

    # Per-core external inputs (host wrapper prepares dtype/layout).
    xb = nc.dram_tensor("xb", [C, HW], BF16, kind="ExternalInput")
    yb = nc.dram_tensor("yb", [C, HW], BF16, kind="ExternalInput")
    zb = nc.dram_tensor("zb", [C, HW], BF16, kind="ExternalInput")
    wqT = nc.dram_tensor("wqT", [C, C], BF16, kind="ExternalInput")   # [ic,oc], pre-scaled 1/sqrt(W)
    wkT = nc.dram_tensor("wkT", [C, C], BF16, kind="ExternalInput")
    wvT = nc.dram_tensor("wvT", [C, C], BF16, kind="ExternalInput")
    bqr = nc.dram_tensor("bqr", [128, C], BF16, kind="ExternalInput")  # bq/sqrt(W) replicated
    bv = nc.dram_tensor("bv", [C, 1], F32, kind="ExternalInput")
    wtap = nc.dram_tensor("wtap", [9 * CH, 128, C], BF16, kind="ExternalInput")  # [tap*2+icc][ic,oc]
    gamma = nc.dram_tensor("gamma", [C, 1], F32, kind="ExternalInput")
    beta = nc.dram_tensor("beta", [C, 1], F32, kind="ExternalInput")

    out = nc.dram_tensor("out", [C, HW], F32, kind="ExternalOutput")

    # DRAM scratch.  v_dram is [h, c, w] so the attention phase's per-16-
    # channel gathers are contiguous (the V-phase writes eat the small-run
    # penalty instead, where DMA has slack).  att_dram is [row, c, x] so the
    # attention phase's stores are contiguous (conv loads eat the penalty
    # under an otherwise PE-bound phase).
    v_dram = nc.dram_tensor("v_scratch", [H, C, W], BF16)
    att_dram = nc.dram_tensor("att_scratch", [H + 2, C, W + 2], BF16)

    with _SplitDrainTileContext(nc) as tc:
        with tc.tile_pool(name="singles", bufs=1) as singles:
            # ---- constants ----
            eps_sb = singles.tile([128, 1], F32)
            nc.vector.memset(eps_sb, BN_EPS)
            zrow = singles.tile([128, CH * (W + 2)], BF16)
            nc.vector.memset(zrow, 0.0)
            # zero the padded top/bottom rows of every att plane
            nc.sync.dma_start(
                out=att_dram[0].rearrange("(p a) w -> p (a w)", p=128), in_=zrow)
            nc.sync.dma_start(
                out=att_dram[H + 1].rearrange("(p a) w -> p (a w)", p=128), in_=zrow)

            wv_sb = singles.tile([128, CH, C], BF16)   # [ic(part), icc, oc]
            nc.sync.dma_start(out=wv_sb, in_=wvT.rearrange("(a p) c -> p a c", p=128))
            bv_sb = singles.tile([128, CH], F32)
            nc.sync.dma_start(out=bv_sb, in_=bv.rearrange("(a p) o -> p (a o)", p=128))

            _mark(nc, 'V')
            # ---------------- Phase V: v = wv@z + bv, channel-major -> DRAM ----
            with tc.tile_pool(name="vz", bufs=3) as vz, \
                 tc.tile_pool(name="vo", bufs=3) as vo, \
                 tc.tile_pool(name="vps", bufs=4, space="PSUM") as vps:
                for vb in range(HW // VB):
                    zt = []
                    for icc in range(CH):
                        t = vz.tile([128, VB], BF16, tag="zt")
                        nc.sync.dma_start(
                            out=t, in_=zb[icc * 128:(icc + 1) * 128, vb * VB:(vb + 1) * VB])
                        zt.append(t)
                    for occ in range(CH):
                        vt = vo.tile([128, VB], BF16, tag="vt")
                        for sub in range(VB // PB):
                            ps = vps.tile([128, PB], F32)
                            for icc in range(CH):
                                nc.tensor.matmul(
                                    ps, lhsT=wv_sb[:, icc, occ * 128:(occ + 1) * 128],
                                    rhs=zt[icc][:, sub * PB:(sub + 1) * PB],
                                    start=(icc == 0), stop=(icc == CH - 1))
                            dst = vt[:, sub * PB:(sub + 1) * PB]
                            if sub % 2 == 0:
                                nc.scalar.activation(
                                    out=dst, in_=ps,
                                    func=mybir.ActivationFunctionType.Identity,
                                    bias=bv_sb[:, occ:occ + 1], scale=1.0)
                            else:
                                nc.vector.tensor_scalar(
                                    out=dst, in0=ps, scalar1=bv_sb[:, occ:occ + 1],
                                    scalar2=None, op0=mybir.AluOpType.add)
                        h0 = (vb * VB) // W
                        nc.sync.dma_start(
                            out=v_dram[h0:h0 + VB // W,
                                       occ * 128:(occ + 1) * 128, :].rearrange(
                                           "h c w -> c h w"),
                            in_=vt)

            _mark(nc, 'QK')
            # ---------------- Phase QK -----------------------------------------
            with tc.tile_pool(name="qk_store", bufs=1) as qkstore:
                Q_sb = qkstore.tile([128, H, C], BF16)   # [w, h, c]
                K_sb = qkstore.tile([128, H, C], BF16)

                wq_sb = singles.tile([128, CH, C], BF16)
                nc.sync.dma_start(out=wq_sb, in_=wqT.rearrange("(a p) c -> p a c", p=128))
                wk_sb = singles.tile([128, CH, C], BF16)
                nc.sync.dma_start(out=wk_sb, in_=wkT.rearrange("(a p) c -> p a c", p=128))
                bq_sb = singles.tile([128, C], BF16)
                nc.sync.dma_start(out=bq_sb, in_=bqr[:, :])

                with tc.tile_pool(name="qkx", bufs=3) as qkx, \
                     tc.tile_pool(name="qkps", bufs=2, space="PSUM") as qkps:
                    for qb in range(HW // QB):
                        xt, yt = [], []
                        for icc in range(CH):
                            t = qkx.tile([128, QB], BF16, tag="xt")
                            nc.sync.dma_start(
                                out=t, in_=xb[icc * 128:(icc + 1) * 128,
                                              qb * QB:(qb + 1) * QB])
                            xt.append(t)
                            t = qkx.tile([128, QB], BF16, tag="yt")
                            nc.sync.dma_start(
                                out=t, in_=yb[icc * 128:(icc + 1) * 128,
                                              qb * QB:(qb + 1) * QB])
                            yt.append(t)
                        for hq in range(QB // (4 * 128)):
                            psQ = qkps.tile([128, 4, C], F32, tag="psQ")
                            psK = qkps.tile([128, 4, C], F32, tag="psK")
                            for jj in range(4):
                                col = (hq * 4 + jj) * 128
                                for sel, (tiles, w_sb, ps) in enumerate(
                                        ((xt, wq_sb, psQ), (yt, wk_sb, psK))):
                                    for icc in range(CH):
                                        nc.tensor.matmul(
                                            ps[:, jj, :],
                                            lhsT=tiles[icc][:, col:col + 128],
                                            rhs=w_sb[:, icc, :],
                                            start=(icc == 0), stop=(icc == CH - 1))
                            h0 = (qb * QB) // 128 + hq * 4
                            # copies: alternate engines to balance load
                            if hq % 2 == 0:
                                nc.vector.tensor_copy(Q_sb[:, h0:h0 + 4, :], psQ)
                                nc.scalar.activation(
                                    out=K_sb[:, h0:h0 + 4, :], in_=psK,
                                    func=mybir.ActivationFunctionType.Identity)
                            else:
                                nc.scalar.activation(
                                    out=Q_sb[:, h0:h0 + 4, :], in_=psQ,
                                    func=mybir.ActivationFunctionType.Identity)
                                nc.vector.tensor_copy(K_sb[:, h0:h0 + 4, :], psK)

                _mark(nc, 'ATT')
                # ---------------- Phase attention, 16 channels per group -------
                with tc.tile_pool(name="att_e", bufs=3) as epool, \
                     tc.tile_pool(name="att_v", bufs=2) as vpool, \
                     tc.tile_pool(name="att_m", bufs=2) as mpool, \
                     tc.tile_pool(name="att_o", bufs=2) as opool, \
                     tc.tile_pool(name="att_r", bufs=3) as rpool, \
                     tc.tile_pool(name="att_sps", bufs=2, space="PSUM") as sps, \
                     tc.tile_pool(name="att_aps", bufs=2, space="PSUM") as aps, \
                     tc.tile_pool(name="att_dps", bufs=2, space="PSUM") as dps, \
                     tc.tile_pool(name="att_skps", bufs=1, space="PSUM") as skps:
                    for g in range(C // GC):
                        c0 = g * GC
                        vt16 = vpool.tile([128, GC, W], BF16, tag="vt16")
                        # gather v[h', c, w] for GC channels in one DMA
                        nc.sync.dma_start(out=vt16, in_=v_dram[:, c0:c0 + GC, :])
                        # m[h', c] = exp(bq_c * sum_w k~[w, h', c])
                        ps_sk = skps.tile([128, GC], F32)
                        for cj in range(GC):
                            nc.tensor.matmul(
                                ps_sk[:, cj:cj + 1], lhsT=K_sb[:, :, c0 + cj],
                                rhs=bq_sb[:, c0 + cj:c0 + cj + 1],
                                start=True, stop=True)
                        m16 = mpool.tile([128, GC], BF16, tag="m16")
                        nc.scalar.activation(
                            out=m16, in_=ps_sk, func=mybir.ActivationFunctionType.Exp)
                        vm16 = vpool.tile([128, GC, W], BF16, tag="vm16")
                        nc.vector.tensor_tensor(
                            out=vm16, in0=vt16,
                            in1=m16.unsqueeze(2).broadcast_to((128, GC, W)),
                            op=mybir.AluOpType.mult)

                        at16 = opool.tile([128, GC, W + 2], BF16, tag="at16")
                        nc.gpsimd.memset(at16[:, :, 0:1], 0.0)
                        nc.gpsimd.memset(at16[:, :, W + 1:W + 2], 0.0)

                        for q4 in range(GC // 4):
                            ps_s = sps.tile([128, 4, H], F32)
                            for j in range(4):
                                c = c0 + q4 * 4 + j
                                nc.tensor.matmul(
                                    ps_s[:, j, :], lhsT=K_sb[:, :, c], rhs=Q_sb[:, :, c],
                                    start=True, stop=True)
                            E4 = epool.tile([128, 4, H], BF16)
                            nc.scalar.activation(
                                out=E4, in_=ps_s, func=mybir.ActivationFunctionType.Exp)
                            ps_a = aps.tile([128, 4, W], F32)
                            ps_d = dps.tile([128, 4], F32)
                            for j in range(4):
                                cj = q4 * 4 + j
                                nc.tensor.matmul(
                                    ps_a[:, j, :], lhsT=E4[:, j, :],
                                    rhs=vm16[:, cj, :], start=True, stop=True)
                                nc.tensor.matmul(
                                    ps_d[:, j:j + 1], lhsT=E4[:, j, :],
                                    rhs=m16[:, cj:cj + 1], start=True, stop=True)
                            r4 = rpool.tile([128, 4], F32)
                            nc.vector.reciprocal(r4, ps_d)
                            dst = at16[:, q4 * 4:q4 * 4 + 4, 1:W + 1]
                            rb = r4.unsqueeze(2).broadcast_to((128, 4, W))
                            nc.vector.tensor_tensor(
                                out=dst, in0=ps_a, in1=rb,
                                op=mybir.AluOpType.mult)
                        nc.sync.dma_start(
                            out=att_dram[1:H + 1, c0:c0 + GC, :], in_=at16)

            _mark(nc, 'CONV')
            # ---------------- Phase conv + BN stats ---------------------------
            with tc.tile_pool(name="conv_store", bufs=1) as cstore:
                o_sb = cstore.tile([128, CH, HW], F32)
                stats_acc = cstore.tile([128, CH, NB, 6], F32)

                wt_sb = singles.tile([128, 9 * CH, C], BF16)
                nc.sync.dma_start(out=wt_sb, in_=wtap.rearrange("t p c -> p t c"))

                with tc.tile_pool(name="conv_in", bufs=3) as cin, \
                     tc.tile_pool(name="conv_ps", bufs=4, space="PSUM") as cps:
                    for cb in range(NB // CB):
                        att_t = []
                        for icc in range(CH):
                            t = cin.tile([128, 4 * CB + 2, W + 2], BF16, tag="att_t")
                            nc.sync.dma_start(
                                out=t,
                                in_=att_dram[4 * CB * cb:4 * CB * cb + 4 * CB + 2,
                                             icc * 128:(icc + 1) * 128, :].rearrange(
                                                 "r c x -> c r x"))
                            att_t.append(t)
                        for pb4 in range(CB):
                            pb = cb * CB + pb4
                            for occ in range(CH):
                                ps = cps.tile([128, PB], F32)
                                n_mm = 9 * CH
                                i_mm = 0
                                for icc in range(CH):
                                    for dy in range(3):
                                        for dx in range(3):
                                            tsel = (dy * 3 + dx) * CH + icc
                                            nc.tensor.matmul(
                                                ps,
                                                lhsT=wt_sb[:, tsel, occ * 128:(occ + 1) * 128],
                                                rhs=att_t[icc][:, 4 * pb4 + dy:4 * pb4 + dy + 4,
                                                               dx:dx + W],
                                                start=(i_mm == 0), stop=(i_mm == n_mm - 1))
                                            i_mm += 1
                                nc.vector.bn_stats(out=stats_acc[:, occ, pb, :], in_=ps)
                                nc.scalar.activation(
                                    out=o_sb[:, occ, pb * PB:(pb + 1) * PB], in_=ps,
                                    func=mybir.ActivationFunctionType.Identity)

                _mark(nc, 'TAIL')
                # ---- finalize stats, AllReduce, apply -------------------------
                g_sb = singles.tile([128, CH], F32)
                nc.sync.dma_start(out=g_sb, in_=gamma.rearrange("(a p) o -> p (a o)", p=128))
                be_sb = singles.tile([128, CH], F32)
                nc.sync.dma_start(out=be_sb, in_=beta.rearrange("(a p) o -> p (a o)", p=128))

                with tc.tile_pool(name="st", bufs=1) as st, \
                     tc.tile_pool(name="st_dram", bufs=1, space="DRAM") as stdram, \
                     tc.tile_pool(name="apply_t", bufs=3) as apool:
                    loc = st.tile([128, 2 * CH], F32)
                    for occ in range(CH):
                        mv = st.tile([128, 2], F32, tag="mv")
                        nc.vector.bn_aggr(out=mv, in_=stats_acc[:, occ])
                        msq = st.tile([128, 1], F32, tag="msq")
                        nc.vector.tensor_mul(msq, mv[:, 0:1], mv[:, 0:1])
                        ex2 = st.tile([128, 1], F32, tag="ex2")
                        nc.vector.tensor_add(ex2, mv[:, 1:2], msq)
                        nc.scalar.mul(out=loc[:, 2 * occ:2 * occ + 1], in_=mv[:, 0:1],
                                      mul=float(HW))
                        nc.scalar.mul(out=loc[:, 2 * occ + 1:2 * occ + 2], in_=ex2,
                                      mul=float(HW))
                    sin = stdram.tile([128, 2 * CH], F32)
                    sout = stdram.tile([128, 2 * CH], F32)
                    nc.gpsimd.dma_start(out=sin, in_=loc)
                    if sim:
                        nc.gpsimd.dma_start(out=sout, in_=sin)
                    else:
                        nc.gpsimd.collective_compute(
                            "AllReduce", mybir.AluOpType.add,
                            replica_groups=[list(range(N_CORES))],
                            ins=[sin.opt()], outs=[sout.opt()])
                    glob = st.tile([128, 2 * CH], F32)
                    nc.gpsimd.dma_start(out=glob, in_=sout)

                    s_t = st.tile([128, CH], F32)
                    t_t = st.tile([128, CH], F32)
                    for occ in range(CH):
                        mg = st.tile([128, 1], F32, tag="mg")
                        nc.scalar.mul(out=mg, in_=glob[:, 2 * occ:2 * occ + 1],
                                      mul=1.0 / N_TOT)
                        e2g = st.tile([128, 1], F32, tag="e2g")
                        nc.scalar.mul(out=e2g, in_=glob[:, 2 * occ + 1:2 * occ + 2],
                                      mul=1.0 / N_TOT)
                        mg2 = st.tile([128, 1], F32, tag="mg2")
                        nc.vector.tensor_mul(mg2, mg, mg)
                        var = st.tile([128, 1], F32, tag="var")
                        nc.vector.tensor_scalar(
                            out=var, in0=e2g, scalar1=mg2, scalar2=None,
                            op0=mybir.AluOpType.subtract)
                        sd = st.tile([128, 1], F32, tag="sd")
                        nc.scalar.activation(
                            out=sd, in_=var, func=mybir.ActivationFunctionType.Sqrt,
                            bias=eps_sb, scale=1.0)
                        rsd = st.tile([128, 1], F32, tag="rsd")
                        nc.vector.reciprocal(rsd, sd)
                        nc.vector.tensor_mul(s_t[:, occ:occ + 1], rsd,
                                             g_sb[:, occ:occ + 1])
                        ms = st.tile([128, 1], F32, tag="ms")
                        nc.vector.tensor_mul(ms, mg, s_t[:, occ:occ + 1])
                        nc.vector.tensor_scalar(
                            out=t_t[:, occ:occ + 1], in0=be_sb[:, occ:occ + 1],
                            scalar1=ms, scalar2=None, op0=mybir.AluOpType.subtract)

                    # y = s*x + t on ACT; LeakyReLU via max(y, 0.2*y) on DVE
                    AB = 2048
                    for occ in range(CH):
                        for ab in range(HW // AB):
                            xin = o_sb[:, occ, ab * AB:(ab + 1) * AB]
                            yt_ = apool.tile([128, AB], F32, tag="yt")
                            nc.scalar.activation(
                                out=yt_, in_=xin,
                                func=mybir.ActivationFunctionType.Identity,
                                scale=s_t[:, occ:occ + 1], bias=t_t[:, occ:occ + 1])
                            ot = apool.tile([128, AB], F32, tag="ot")
                            nc.vector.scalar_tensor_tensor(
                                out=ot, in0=yt_, scalar=LEAKY, in1=yt_,
                                op0=mybir.AluOpType.mult, op1=mybir.AluOpType.max)
                            nc.sync.dma_start(
                                out=out[occ * 128:(occ + 1) * 128, ab * AB:(ab + 1) * AB],
                                in_=ot)
    return nc


_NC_CACHE = None


def _get_nc():
    global _NC_CACHE
    if _NC_CACHE is None:
        _NC_CACHE = _build()
    return _NC_CACHE


def kernel(x, y, z, wq, bq, wk, bk, wv, bv, w_out, b_out, gamma, beta, **_unused):
    x = np.asarray(x, dtype=np.float32)
    y = np.asarray(y, dtype=np.float32)
    z = np.asarray(z, dtype=np.float32)
    scale = 1.0 / math.sqrt(W)

    wqT = np.ascontiguousarray((np.asarray(wq, np.float32).T * scale).astype(nbf16))
    wkT = np.ascontiguousarray(np.asarray(wk, np.float32).T.astype(nbf16))
    wvT = np.ascontiguousarray(np.asarray(wv, np.float32).T.astype(nbf16))
    bqr = np.broadcast_to((np.asarray(bq, np.float32) * scale).reshape(1, C),
                          (128, C)).astype(nbf16)
    bvh = np.asarray(bv, np.float32).reshape(C, 1)
    # w_out [oc, ic, 3, 3] -> wtap[(dy*3+dx)*CH + icc][ic(128), oc]
    wo = np.asarray(w_out, np.float32)
    wtap = np.empty((9 * CH, 128, C), dtype=nbf16)
    for dy in range(3):
        for dx in range(3):
            wt = wo[:, :, dy, dx].T  # [ic, oc]
            for icc in range(CH):
                wtap[(dy * 3 + dx) * CH + icc] = wt[icc * 128:(icc + 1) * 128].astype(nbf16)
    gm = np.asarray(gamma, np.float32).reshape(C, 1)
    bt = np.asarray(beta, np.float32).reshape(C, 1)

    shared = dict(wqT=wqT, wkT=wkT, wvT=wvT, bqr=bqr, bv=bvh,
                  wtap=wtap, gamma=gm, beta=bt)
    in_maps = []
    for i in range(N_CORES):
        in_maps.append(dict(
            xb=x[i].reshape(C, HW).astype(nbf16),
            yb=y[i].reshape(C, HW).astype(nbf16),
            zb=z[i].reshape(C, HW).astype(nbf16),
            **shared))

    nc = _get_nc()
    global _last_in_maps
    _last_in_maps = in_maps
    res = run_bass_kernel_spmd(nc, in_maps, list(range(N_CORES)))
    out = np.stack([res.results[i]["out"].reshape(C, H, W) for i in range(N_CORES)])
    return out.astype(np.float32)


if __name__ == "__main__":
    pass


# revision 15
# speedup vs baseline: 17.9306x; 17.9306x over previous
"""Trainium2 Bass kernel for nn_MultiHeadedAttention_41583873359904.

Reference computation (B=8, C=256, H=W=128):
  q/k/v = 1x1 conv projections of x/y/z
  scores[b,c,h,h'] = q[b,c,h,:].k[b,c,h',:]/sqrt(W); p = softmax(scores, -1)
  att = p @ v  (per b,c)
  o = conv3x3(att) + b_out -> BatchNorm2d(batch stats) -> LeakyReLU(0.2)

Sharding: data-parallel over batch, one batch element per NeuronCore (8 cores).
BatchNorm batch stats are combined with an on-device AllReduce of per-core
(sum, sumsq) so the whole computation is a single NEFF.

Bias handling (all exact):
  - b_out cancels through BatchNorm (constant per channel, subtracted by mean).
  - bk adds only h-dependent terms to scores -> cancels in softmax over h'.
  - bq adds bq*Sk(h') to scores (Sk = sum_w k~): folded multiplicatively as
    m[h'] = exp(bq*Sk[h']); att = (E @ (m*v)) / (E @ m). Sk comes from one
    N=1 matmul per channel against a bq-replicated column, m from one Exp.
  - bv is applied in the V projection via the activation bias operand.

Per-core layout strategy (DMAs are batched aggressively -- each dma_start
costs ~600ns on the shared descriptor generator regardless of size):
  - V projection channel-major [oc, pix] -> v_dram[C, HW]; attention reads it
    back as [h', 16ch, w] gather tiles (one DMA per 16 channels).
  - Q/K projections pixel-major -> Q_sb/K_sb [w, h, c] in SBUF (contraction
    over w needs w on partitions); scores^T = K^T.T @ Q^T per channel,
    Exp on ACT, att = E^T.T @ (m*v) with an m-column matmul giving the
    softmax denominator; normalize folded into the PSUM->SBUF copy.
  - att planes written zero-padded [C,130,130] to DRAM in 16-channel blocks;
    3x3 conv = 18 accumulated matmuls (2 ic chunks x 9 taps) per
    [128oc, 512pix] PSUM tile with shifted access patterns.

Matmul operands are bf16 (fp32 PSUM accumulation).
"""

import math

import numpy as np
import ml_dtypes

import concourse.bass as bass
import concourse.tile as tile
from concourse import mybir
from concourse import tile_sem_assignment as _tsa
from concourse.tile import ScopedClock as _ScopedClock
from concourse.bass_utils import run_bass_kernel_spmd

B, C, H, W = 8, 256, 128, 128
HW = H * W          # 16384 pixels per plane
PB = 512            # pixels per conv/proj psum tile (4 rows)
NB = HW // PB       # 32 pixel blocks
CH = C // 128       # 2 channel chunks of 128
GC = 16             # channels per attention group
BN_EPS = 1e-5
LEAKY = 0.2
N_CORES = 8
N_TOT = float(B * HW)   # BN element count per channel

VB = 2048           # pixels per V-phase chunk (8 chunks)
QB = 2048           # pixels per QK-phase chunk (16 h-rows)
CB = 4              # conv pb blocks per load (16 h-rows)

BF16 = mybir.dt.bfloat16
F32 = mybir.dt.float32
nbf16 = ml_dtypes.bfloat16


class _SplitDrainTileContext(tile.TileContext):
    """The walrus in this container rejects >1 sync wait per instruction.
    Tile routinely emits several (RAW + WAR). Hoist extra waits onto NOPs
    committed immediately before on the same engine (sequencers execute in
    order, so waiting on the NOPs first is equivalent), and split the tail
    drain's global-clock waits the same way."""

    def _commit_instruction(self, inst, lazy_reg_writes=True):
        si = getattr(inst, "sync_info", None)
        if (
            si is not None
            and si.on_wait
            and len(si.on_wait) > 1
            and inst.engine != mybir.EngineType.Unassigned
            and not isinstance(inst, mybir.InstUnconditionalBranch)
        ):
            waits = list(si.on_wait)
            for w in waits[:-1]:
                nop = mybir.InstNoOp(
                    name=self.nc.get_next_instruction_name(),
                    engine=inst.engine,
                    ins=[],
                    outs=[],
                    sync_info=mybir.SyncInfo(on_wait=[w], on_update=[]),
                    bass_nofuse=True,
                )
                super()._commit_instruction(nop, lazy_reg_writes=False)
            inst.sync_info = mybir.SyncInfo(
                on_wait=[waits[-1]], on_update=list(si.on_update or [])
            )
        super()._commit_instruction(inst, lazy_reg_writes)

    def _drain_and_barrier(self, tick_clock, wait_clock):
        nc = self.nc
        gc = tick_clock.global_clock
        procs = [(p, gc.peek_next(p) - 1) for p in range(_tsa.N_PROCS)]
        for p, t in procs:
            if t <= 0:
                continue
            sub = _tsa.VectorClock()
            sub.require_at_least(p, t)
            nop = nc.sync.nop(nofuse=True, hint="split_drain_wait")
            wait_clock.add_sem_waits(nop.ins, _ScopedClock({None: sub}))
        nc.sync.drain()
        nc.all_engine_barrier()
        assert self.sems is not None
        popped = nc._tile_sem_poison_stack.pop()
        assert popped is self._sem_poison
        nc.clear_and_free_semaphores(list(self.sems.allocated().values()))
        nc.all_engine_barrier()


_PHASE_MARKS = []


def _mark(nc, name):
    _PHASE_MARKS.append((name, int(nc.get_next_instruction_name()[2:])))


def _build(sim=False, qbias=True):
    _PHASE_MARKS.clear()
    nc = bass.Bass(num_devices=N_CORES)

    # Per-core external inputs (host wrapper prepares dtype/layout).
    xb = nc.dram_tensor("xb", [C, HW], BF16, kind="ExternalInput")
    yb = nc.dram_tensor("yb", [C, HW], BF16, kind="ExternalInput")
    zb = nc.dram_tensor("zb", [C, HW], BF16, kind="ExternalInput")
    wqT = nc.dram_tensor("wqT", [C, C], BF16, kind="ExternalInput")   # [ic,oc], pre-scaled 1/sqrt(W)
    wkT = nc.dram_tensor("wkT", [C, C], BF16, kind="ExternalInput")
    wvT = nc.dram_tensor("wvT", [C, C], BF16, kind="ExternalInput")
    bqr = (nc.dram_tensor("bqr", [128, C], BF16, kind="ExternalInput")
           if qbias else None)  # bq/sqrt(W) replicated
    bv = nc.dram_tensor("bv", [C, 1], F32, kind="ExternalInput")
    wtap = nc.dram_tensor("wtap", [9 * CH, 128, C], BF16, kind="ExternalInput")  # [tap*2+icc][ic,oc]
    gamma = nc.dram_tensor("gamma", [C, 1], F32, kind="ExternalInput")
    beta = nc.dram_tensor("beta", [C, 1], F32, kind="ExternalInput")

    out = nc.dram_tensor("out", [C, HW], F32, kind="ExternalOutput")

    # DRAM scratch.  v_dram is [h, c, w] so the attention phase's per-16-
    # channel gathers are contiguous (the V-phase writes eat the small-run
    # penalty instead, where DMA has slack).  att_dram is [row, c, x] so the
    # attention phase's stores are contiguous (conv loads eat the penalty
    # under an otherwise PE-bound phase).
    v_dram = nc.dram_tensor("v_scratch", [C, HW], BF16)
    att_dram = nc.dram_tensor("att_scratch", [H + 2, C, W + 2], BF16)

    with _SplitDrainTileContext(nc) as tc:
        with tc.tile_pool(name="singles", bufs=1) as singles:
            # ---- constants ----
            eps_sb = singles.tile([128, 1], F32)
            nc.vector.memset(eps_sb, BN_EPS)
            ones_col = singles.tile([128, 1], BF16)
            nc.vector.memset(ones_col, 1.0)
            zrow = singles.tile([128, CH * (W + 2)], BF16)
            nc.vector.memset(zrow, 0.0)
            # zero the padded top/bottom rows of every att plane
            nc.sync.dma_start(
                out=att_dram[0].rearrange("(p a) w -> p (a w)", p=128), in_=zrow)
            nc.sync.dma_start(
                out=att_dram[H + 1].rearrange("(p a) w -> p (a w)", p=128), in_=zrow)

            wv_sb = singles.tile([128, CH, C], BF16)   # [ic(part), icc, oc]
            nc.sync.dma_start(out=wv_sb, in_=wvT.rearrange("(a p) c -> p a c", p=128))
            bv_sb = singles.tile([128, CH], F32)
            nc.sync.dma_start(out=bv_sb, in_=bv.rearrange("(a p) o -> p (a o)", p=128))

            _mark(nc, 'V')
            # ------- Phases V and QK, interleaved per 2048-pixel chunk --------
            # V (v = wv@z + bv, channel-major -> DRAM) is DMA-heavy / PE-light;
            # QK (pixel-major projections into SBUF) is PE-heavy.  Issuing one
            # chunk of each per iteration lets the engines hide V's transfers
            # under QK's matmuls.
            with tc.tile_pool(name="qk_store", bufs=1) as qkstore:
                Q_sb = qkstore.tile([128, H, C], BF16)   # [w, h, c]
                K_sb = qkstore.tile([128, H, C], BF16)

                wq_sb = singles.tile([128, CH, C], BF16)
                nc.sync.dma_start(out=wq_sb, in_=wqT.rearrange("(a p) c -> p a c", p=128))
                wk_sb = singles.tile([128, CH, C], BF16)
                nc.sync.dma_start(out=wk_sb, in_=wkT.rearrange("(a p) c -> p a c", p=128))
                if qbias:
                    bq_sb = singles.tile([128, C], BF16)
                    nc.sync.dma_start(out=bq_sb, in_=bqr[:, :])

                with tc.tile_pool(name="vz", bufs=2) as vz, \
                     tc.tile_pool(name="vo", bufs=2) as vo, \
                     tc.tile_pool(name="vps", bufs=2, space="PSUM") as vps, \
                     tc.tile_pool(name="qkx", bufs=2) as qkx, \
                     tc.tile_pool(name="qkps", bufs=3, space="PSUM") as qkps:
                    for vb in range(HW // VB):
                        # ---- V chunk ----
                        zt = []
                        for icc in range(CH):
                            t = vz.tile([128, VB], BF16, tag="zt")
                            nc.sync.dma_start(
                                out=t, in_=zb[icc * 128:(icc + 1) * 128,
                                              vb * VB:(vb + 1) * VB])
                            zt.append(t)
                        for occ in range(CH):
                            vt = vo.tile([128, VB], BF16, tag="vt")
                            for sub in range(VB // PB):
                                ps = vps.tile([128, PB], F32)
                                for icc in range(CH):
                                    nc.tensor.matmul(
                                        ps, lhsT=wv_sb[:, icc, occ * 128:(occ + 1) * 128],
                                        rhs=zt[icc][:, sub * PB:(sub + 1) * PB],
                                        start=(icc == 0), stop=(icc == CH - 1))
                                dst = vt[:, sub * PB:(sub + 1) * PB]
                                if sub % 2 == 0:
                                    nc.scalar.activation(
                                        out=dst, in_=ps,
                                        func=mybir.ActivationFunctionType.Identity,
                                        bias=bv_sb[:, occ:occ + 1], scale=1.0)
                                else:
                                    nc.vector.tensor_scalar(
                                        out=dst, in0=ps, scalar1=bv_sb[:, occ:occ + 1],
                                        scalar2=None, op0=mybir.AluOpType.add)
                            nc.gpsimd.dma_start(
                                out=v_dram[occ * 128:(occ + 1) * 128,
                                           vb * VB:(vb + 1) * VB],
                                in_=vt)

                        # ---- QK chunk ----
                        qb = vb
                        xt, yt = [], []
                        for icc in range(CH):
                            t = qkx.tile([128, QB], BF16, tag="xt")
                            nc.sync.dma_start(
                                out=t, in_=xb[icc * 128:(icc + 1) * 128,
                                              qb * QB:(qb + 1) * QB])
                            xt.append(t)
                            t = qkx.tile([128, QB], BF16, tag="yt")
                            nc.sync.dma_start(
                                out=t, in_=yb[icc * 128:(icc + 1) * 128,
                                              qb * QB:(qb + 1) * QB])
                            yt.append(t)
                        for hq in range(QB // (2 * 128)):
                            psQ = qkps.tile([128, 2, C], F32, tag="psQ")
                            psK = qkps.tile([128, 2, C], F32, tag="psK")
                            for jj in range(2):
                                col = (hq * 2 + jj) * 128
                                for sel, (tiles, w_sb, ps) in enumerate(
                                        ((xt, wq_sb, psQ), (yt, wk_sb, psK))):
                                    for icc in range(CH):
                                        nc.tensor.matmul(
                                            ps[:, jj, :],
                                            lhsT=tiles[icc][:, col:col + 128],
                                            rhs=w_sb[:, icc, :],
                                            start=(icc == 0), stop=(icc == CH - 1))
                            h0 = (qb * QB) // 128 + hq * 2
                            # copies: alternate engines to balance load
                            if hq % 2 == 0:
                                nc.vector.tensor_copy(Q_sb[:, h0:h0 + 2, :], psQ)
                                nc.scalar.activation(
                                    out=K_sb[:, h0:h0 + 2, :], in_=psK,
                                    func=mybir.ActivationFunctionType.Identity)
                            else:
                                nc.scalar.activation(
                                    out=Q_sb[:, h0:h0 + 2, :], in_=psQ,
                                    func=mybir.ActivationFunctionType.Identity)
                                nc.vector.tensor_copy(K_sb[:, h0:h0 + 2, :], psK)

                _mark(nc, 'ATT')
                # ---------------- Phase attention, 16 channels per group -------
                with tc.tile_pool(name="att_e", bufs=3) as epool, \
                     tc.tile_pool(name="att_v", bufs=2) as vpool, \
                     tc.tile_pool(name="att_m", bufs=2) as mpool, \
                     tc.tile_pool(name="att_o", bufs=2) as opool, \
                     tc.tile_pool(name="att_r", bufs=3) as rpool, \
                     tc.tile_pool(name="att_sps", bufs=2, space="PSUM") as sps, \
                     tc.tile_pool(name="att_aps", bufs=2, space="PSUM") as aps, \
                     tc.tile_pool(name="att_dps", bufs=2, space="PSUM") as dps, \
                     tc.tile_pool(name="att_skps", bufs=1, space="PSUM") as skps:
                    for g in range(C // GC):
                        c0 = g * GC
                        vt16 = vpool.tile([128, GC, W], BF16, tag="vt16")
                        # gather v[h', c, w] for GC channels in one DMA
                        nc.sync.dma_start(
                            out=vt16,
                            in_=v_dram[c0:c0 + GC, :].rearrange(
                                "c (h w) -> h c w", w=W))
                        if qbias:
                            # m[h', c] = exp(bq_c * sum_w k~[w, h', c])
                            ps_sk = skps.tile([128, GC], F32)
                            for cj in range(GC):
                                nc.tensor.matmul(
                                    ps_sk[:, cj:cj + 1], lhsT=K_sb[:, :, c0 + cj],
                                    rhs=bq_sb[:, c0 + cj:c0 + cj + 1],
                                    start=True, stop=True)
                            m16 = mpool.tile([128, GC], BF16, tag="m16")
                            nc.scalar.activation(
                                out=m16, in_=ps_sk,
                                func=mybir.ActivationFunctionType.Exp)
                            vm16 = vpool.tile([128, GC, W], BF16, tag="vm16")
                            nc.vector.tensor_tensor(
                                out=vm16, in0=vt16,
                                in1=m16.unsqueeze(2).broadcast_to((128, GC, W)),
                                op=mybir.AluOpType.mult)
                        else:
                            vm16 = vt16

                        at16 = opool.tile([128, GC, W + 2], BF16, tag="at16")
                        nc.vector.memset(at16[:, :, 0:1], 0.0)
                        nc.vector.memset(at16[:, :, W + 1:W + 2], 0.0)

                        for q4 in range(GC // 4):
                            ps_s = sps.tile([128, 4, H], F32)
                            for j in range(4):
                                c = c0 + q4 * 4 + j
                                nc.tensor.matmul(
                                    ps_s[:, j, :], lhsT=K_sb[:, :, c], rhs=Q_sb[:, :, c],
                                    start=True, stop=True)
                            E4 = epool.tile([128, 4, H], BF16)
                            nc.scalar.activation(
                                out=E4, in_=ps_s, func=mybir.ActivationFunctionType.Exp)
                            ps_a = aps.tile([128, 4, W], F32)
                            ps_d = dps.tile([128, 4], F32)
                            for j in range(4):
                                cj = q4 * 4 + j
                                nc.tensor.matmul(
                                    ps_a[:, j, :], lhsT=E4[:, j, :],
                                    rhs=vm16[:, cj, :], start=True, stop=True)
                                nc.tensor.matmul(
                                    ps_d[:, j:j + 1], lhsT=E4[:, j, :],
                                    rhs=(m16[:, cj:cj + 1] if qbias else ones_col),
                                    start=True, stop=True)
                            r4 = rpool.tile([128, 4], F32)
                            nc.vector.reciprocal(r4, ps_d)
                            dst = at16[:, q4 * 4:q4 * 4 + 4, 1:W + 1]
                            rb = r4.unsqueeze(2).broadcast_to((128, 4, W))
                            nc.vector.tensor_tensor(
                                out=dst, in0=ps_a, in1=rb,
                                op=mybir.AluOpType.mult)
                        nc.gpsimd.dma_start(
                            out=att_dram[1:H + 1, c0:c0 + GC, :], in_=at16)

            _mark(nc, 'CONV')
            # ------- Phase conv + BN stats (occ-outer; apply[0] overlaps occ=1)
            with tc.tile_pool(name="conv_store", bufs=1) as cstore:
                o_sb = cstore.tile([128, CH, HW], BF16)
                stats_acc = cstore.tile([128, CH, NB, 6], F32)

                wt_sb = singles.tile([128, 9 * CH, C], BF16)
                nc.sync.dma_start(out=wt_sb, in_=wtap.rearrange("t p c -> p t c"))
                g_sb = singles.tile([128, CH], F32)
                nc.sync.dma_start(out=g_sb, in_=gamma.rearrange("(a p) o -> p (a o)", p=128))
                be_sb = singles.tile([128, CH], F32)
                nc.sync.dma_start(out=be_sb, in_=beta.rearrange("(a p) o -> p (a o)", p=128))

                AB = 2048
                with tc.tile_pool(name="conv_in", bufs=1) as cin, \
                     tc.tile_pool(name="st", bufs=1) as st, \
                     tc.tile_pool(name="st_dram", bufs=1, space="DRAM") as stdram, \
                     tc.tile_pool(name="apply_t", bufs=3) as apool, \
                     tc.tile_pool(name="conv_ps", bufs=4, space="PSUM") as cps:
                    s_t = st.tile([128, CH], F32)
                    t_t = st.tile([128, CH], F32)

                    def apply_chunk(occ, ab):
                        xin = o_sb[:, occ, ab * AB:(ab + 1) * AB]
                        yt_ = apool.tile([128, AB], F32, tag="yt")
                        nc.scalar.activation(
                            out=yt_, in_=xin,
                            func=mybir.ActivationFunctionType.Identity,
                            scale=s_t[:, occ:occ + 1], bias=t_t[:, occ:occ + 1])
                        ot = apool.tile([128, AB], F32, tag="ot")
                        nc.vector.scalar_tensor_tensor(
                            out=ot, in0=yt_, scalar=LEAKY, in1=yt_,
                            op0=mybir.AluOpType.mult, op1=mybir.AluOpType.max)
                        nc.gpsimd.dma_start(
                            out=out[occ * 128:(occ + 1) * 128, ab * AB:(ab + 1) * AB],
                            in_=ot)

                    att_res = {}
                    for occ in range(CH):
                        for cb in range(NB // CB):
                            if occ == 0:
                                att_t = []
                                for icc in range(CH):
                                    t = cin.tile([128, 4 * CB + 2, W + 2], BF16,
                                                 tag=f"att_t{cb}_{icc}")
                                    nc.sync.dma_start(
                                        out=t,
                                        in_=att_dram[4 * CB * cb:4 * CB * cb + 4 * CB + 2,
                                                     icc * 128:(icc + 1) * 128, :].rearrange(
                                                         "r c x -> c r x"))
                                    att_t.append(t)
                                att_res[cb] = att_t
                            else:
                                att_t = att_res[cb]
                            for pb4 in range(CB):
                                pb = cb * CB + pb4
                                ps = cps.tile([128, PB], F32)
                                n_mm = 9 * CH
                                i_mm = 0
                                for icc in range(CH):
                                    for dy in range(3):
                                        for dx in range(3):
                                            tsel = (dy * 3 + dx) * CH + icc
                                            nc.tensor.matmul(
                                                ps,
                                                lhsT=wt_sb[:, tsel, occ * 128:(occ + 1) * 128],
                                                rhs=att_t[icc][:, 4 * pb4 + dy:4 * pb4 + dy + 4,
                                                               dx:dx + W],
                                                start=(i_mm == 0), stop=(i_mm == n_mm - 1))
                                            i_mm += 1
                                nc.vector.bn_stats(out=stats_acc[:, occ, pb, :], in_=ps)
                                nc.scalar.activation(
                                    out=o_sb[:, occ, pb * PB:(pb + 1) * PB], in_=ps,
                                    func=mybir.ActivationFunctionType.Identity)
                            if occ == 1:
                                apply_chunk(0, cb)

                        # ---- per-occ stats finalize + AllReduce ----------------
                        mv = st.tile([128, 2], F32, tag=f"mv{occ}")
                        nc.vector.bn_aggr(out=mv, in_=stats_acc[:, occ])
                        msq = st.tile([128, 1], F32, tag=f"msq{occ}")
                        nc.vector.tensor_mul(msq, mv[:, 0:1], mv[:, 0:1])
                        ex2 = st.tile([128, 1], F32, tag=f"ex2{occ}")
                        nc.vector.tensor_add(ex2, mv[:, 1:2], msq)
                        loc = st.tile([128, 2], F32, tag=f"loc{occ}")
                        nc.scalar.mul(out=loc[:, 0:1], in_=mv[:, 0:1], mul=float(HW))
                        nc.scalar.mul(out=loc[:, 1:2], in_=ex2, mul=float(HW))
                        sin = stdram.tile([128, 2], F32, tag=f"sin{occ}")
                        sout = stdram.tile([128, 2], F32, tag=f"sout{occ}")
                        nc.gpsimd.dma_start(out=sin, in_=loc)
                        if sim:
                            nc.gpsimd.dma_start(out=sout, in_=sin)
                        else:
                            nc.gpsimd.collective_compute(
                                "AllReduce", mybir.AluOpType.add,
                                replica_groups=[list(range(N_CORES))],
                                ins=[sin.opt()], outs=[sout.opt()])
                        glob = st.tile([128, 2], F32, tag=f"glob{occ}")
                        nc.gpsimd.dma_start(out=glob, in_=sout)

                        mg = st.tile([128, 1], F32, tag=f"mg{occ}")
                        nc.scalar.mul(out=mg, in_=glob[:, 0:1], mul=1.0 / N_TOT)
                        e2g = st.tile([128, 1], F32, tag=f"e2g{occ}")
                        nc.scalar.mul(out=e2g, in_=glob[:, 1:2], mul=1.0 / N_TOT)
                        mg2 = st.tile([128, 1], F32, tag=f"mg2{occ}")
                        nc.vector.tensor_mul(mg2, mg, mg)
                        var = st.tile([128, 1], F32, tag=f"var{occ}")
                        nc.vector.tensor_scalar(
                            out=var, in0=e2g, scalar1=mg2, scalar2=None,
                            op0=mybir.AluOpType.subtract)
                        sd = st.tile([128, 1], F32, tag=f"sd{occ}")
                        nc.scalar.activation(
                            out=sd, in_=var, func=mybir.ActivationFunctionType.Sqrt,
                            bias=eps_sb, scale=1.0)
                        rsd = st.tile([128, 1], F32, tag=f"rsd{occ}")
                        nc.vector.reciprocal(rsd, sd)
                        nc.vector.tensor_mul(s_t[:, occ:occ + 1], rsd,
                                             g_sb[:, occ:occ + 1])
                        ms = st.tile([128, 1], F32, tag=f"ms{occ}")
                        nc.vector.tensor_mul(ms, mg, s_t[:, occ:occ + 1])
                        nc.vector.tensor_scalar(
                            out=t_t[:, occ:occ + 1], in0=be_sb[:, occ:occ + 1],
                            scalar1=ms, scalar2=None, op0=mybir.AluOpType.subtract)

                    _mark(nc, 'TAIL')
                    for ab in range(HW // AB):
                        apply_chunk(1, ab)
    return nc


_NC_CACHE = {}


def _get_nc(qbias=False):
    if qbias not in _NC_CACHE:
        _NC_CACHE[qbias] = _build(qbias=qbias)
    return _NC_CACHE[qbias]


def kernel(x, y, z, wq, bq, wk, bk, wv, bv, w_out, b_out, gamma, beta, **_unused):
    x = np.asarray(x, dtype=np.float32)
    y = np.asarray(y, dtype=np.float32)
    z = np.asarray(z, dtype=np.float32)
    scale = 1.0 / math.sqrt(W)

    wqT = np.ascontiguousarray((np.asarray(wq, np.float32).T * scale).astype(nbf16))
    wkT = np.ascontiguousarray(np.asarray(wk, np.float32).T.astype(nbf16))
    wvT = np.ascontiguousarray(np.asarray(wv, np.float32).T.astype(nbf16))
    bqr = np.broadcast_to((np.asarray(bq, np.float32) * scale).reshape(1, C),
                          (128, C)).astype(nbf16)
    bvh = np.asarray(bv, np.float32).reshape(C, 1)
    # w_out [oc, ic, 3, 3] -> wtap[(dy*3+dx)*CH + icc][ic(128), oc]
    wo = np.asarray(w_out, np.float32)
    wtap = np.empty((9 * CH, 128, C), dtype=nbf16)
    for dy in range(3):
        for dx in range(3):
            wt = wo[:, :, dy, dx].T  # [ic, oc]
            for icc in range(CH):
                wtap[(dy * 3 + dx) * CH + icc] = wt[icc * 128:(icc + 1) * 128].astype(nbf16)
    gm = np.asarray(gamma, np.float32).reshape(C, 1)
    bt = np.asarray(beta, np.float32).reshape(C, 1)

    shared = dict(wqT=wqT, wkT=wkT, wvT=wvT, bqr=bqr, bv=bvh,
                  wtap=wtap, gamma=gm, beta=bt)
    in_maps = []
    for i in range(N_CORES):
        in_maps.append(dict(
            xb=x[i].reshape(C, HW).astype(nbf16),
            yb=y[i].reshape(C, HW).astype(nbf16),
            zb=z[i].reshape(C, HW).astype(nbf16),
            **shared))

    qbias = bool(np.any(np.asarray(bq, np.float32) != 0.0))
    nc = _get_nc(qbias=qbias)
    global _last_in_maps
    _last_in_maps = in_maps
    res = run_bass_kernel_spmd(nc, in_maps, list(range(N_CORES)))
    out = np.stack([res.results[i]["out"].reshape(C, H, W) for i in range(N_CORES)])
    return out.astype(np.float32)


if __name__ == "__main__":
    pass
